# revision 13
# baseline (speedup 1.0000x reference)
# SlotAttention Trainium2 kernel: data-parallel over batch across 8 NeuronCores.
# Each core processes 2 batches fully on-chip: k/v projections are built once
# in bf16 SBUF-resident tensors, then 3 slot-attention iterations run without
# touching HBM (softmax is over the 8 slots, so the attention streams in a
# single pass per iteration with PSUM-accumulated updates).
import numpy as np
import ml_dtypes

import concourse.bass as bass
from concourse import bacc
import concourse.tile as tile
from concourse import mybir
from concourse.bass_utils import run_bass_kernel_spmd
from concourse.masks import make_identity

F32 = mybir.dt.float32
BF16 = mybir.dt.bfloat16
AFT = mybir.ActivationFunctionType
AX = mybir.AxisListType
ALU = mybir.AluOpType
NPBF16 = ml_dtypes.bfloat16

B, NQ, NS, D, H = 16, 8, 16384, 128, 512
NCORES = 8
BPC = B // NCORES          # batches per core
ITERS = 3
LN_EPS = 1e-5
SCALE = D ** -0.5
CHUNK = 128
NCH = NS // CHUNK          # 128 chunks per batch
G = 16                     # chunks per softmax group
NGRP = NCH // G

PARAM_SPECS = [
    ("wkT", (D, D), BF16), ("wvT", (D, D), BF16), ("wqT", (D, D), BF16),
    ("wiT", (D, 3 * D), BF16), ("whT", (D, 3 * D), BF16),
    ("w1T", (D, H), BF16), ("w2T", (D, 4, D), BF16),
    ("bkcol", (D, 1), F32), ("bqcol", (D, 1), F32), ("bvrep", (D, D), F32),
    ("big8", (2 * NQ, 2 * D), F32), ("bin8", (2 * NQ, D), F32),
    ("bhn8", (2 * NQ, D), F32), ("b18", (2 * NQ, H), F32),
    ("b28", (2 * NQ, D), F32),
]


def _layernorm_to(nc, work, pool_eps, src, p, out_dtype, tag):
    """(src - mean) * rsqrt(var + eps) -> new [p, D] tile (affine folded into
    the downstream projection weights host-side)."""
    st = work.tile([p, 6], F32, tag=tag + "_st")
    nc.vector.bn_stats(out=st, in_=src)
    mv = work.tile([p, 2], F32, tag=tag + "_mv")
    nc.vector.bn_aggr(out=mv, in_=st)
    sd = work.tile([p, 1], F32, tag=tag + "_sd")
    nc.scalar.activation(out=sd, in_=mv[:, 1:2], func=AFT.Sqrt, bias=pool_eps[:p])
    nc.vector.reciprocal(out=sd, in_=sd)
    xn = work.tile([p, D], out_dtype, tag=tag + "_xn")
    nc.vector.tensor_scalar(
        out=xn, in0=src, scalar1=mv[:, 0:1], scalar2=sd,
        op0=ALU.subtract, op1=ALU.mult,
    )
    return xn


def _build_bass():
    nc = bacc.Bacc("TRN2", debug=False)
    inp = nc.dram_tensor("inp", (BPC, NS, D), F32, kind="ExternalInput")[:]
    qry = nc.dram_tensor("qry", (BPC * NQ, D), F32, kind="ExternalInput")[:]
    prm = {
        name: nc.dram_tensor(name, shape, dt, kind="ExternalInput")[:]
        for name, shape, dt in PARAM_SPECS
    }
    out = nc.dram_tensor("out", (BPC * NQ, D), F32, kind="ExternalOutput")[:]

    with tile.TileContext(nc) as tc:
        with (
            tc.tile_pool(name="singles", bufs=1) as singles,
            tc.tile_pool(name="work", bufs=3) as work,
        ):
            ident = singles.tile([128, 128], BF16)
            make_identity(nc, ident)
            eps_t = singles.tile([128, 1], F32)
            nc.vector.memset(eps_t, LN_EPS)

            sb = {}
            for name, shape, dt in PARAM_SPECS:
                t = singles.tile(list(shape), dt, name=name)
                nc.sync.dma_start(out=t, in_=prm[name])
                sb[name] = t

            kT = [singles.tile([D, NS], BF16, tag=f"kT{b}", name=f"kT{b}") for b in range(BPC)]
            vau = [
                singles.tile([CHUNK, NCH, CHUNK + 1], BF16, tag=f"vau{b}", name=f"vau{b}")
                for b in range(BPC)
            ]
            for b in range(BPC):
                nc.vector.memset(vau[b][:, :, CHUNK:CHUNK + 1], 1.0)

            # ---------------- build k/v ----------------
            # 512-token groups: one DMA + one bn_stats/aggr per group; LN
            # apply runs per 128-token chunk on the otherwise-idle GPSIMD.
            S = 4
            with tc.tile_pool(name="kvps", bufs=2, space="PSUM") as kvps:
                for b in range(BPC):
                    for g4 in range(NCH // S):
                        xg = work.tile([CHUNK, S, D], F32, tag="xg")
                        src = inp[
                            b, g4 * S * CHUNK:(g4 + 1) * S * CHUNK, :
                        ].rearrange("(s p) e -> p s e", p=CHUNK)
                        nc.sync.dma_start(out=xg, in_=src)
                        st = work.tile([CHUNK, S, 6], F32, tag="bst")
                        mv = work.tile([CHUNK, S, 2], F32, tag="bmv")
                        for s in range(S):
                            nc.vector.bn_stats(out=st[:, s, :], in_=xg[:, s, :])
                            nc.vector.bn_aggr(out=mv[:, s, :], in_=st[:, s, :])
                        sd = work.tile([CHUNK, S], F32, tag="bsd")
                        nc.scalar.activation(
                            out=sd, in_=mv[:, :, 1], func=AFT.Sqrt, bias=eps_t
                        )
                        nc.vector.reciprocal(out=sd, in_=sd)
                        for s in range(S):
                            c = g4 * S + s
                            xn = work.tile([CHUNK, D], BF16, tag="xn")
                            nc.gpsimd.tensor_scalar(
                                out=xn, in0=xg[:, s, :],
                                scalar1=mv[:, s, 0:1], scalar2=sd[:, s:s + 1],
                                op0=ALU.subtract, op1=ALU.mult,
                            )
                            xnT_ps = kvps.tile([D, CHUNK], BF16, tag="xTps")
                            nc.tensor.transpose(xnT_ps, xn, ident)
                            xnT = work.tile([D, CHUNK], BF16, tag="xnT")
                            nc.vector.tensor_copy(out=xnT, in_=xnT_ps)
                            kps = kvps.tile([D, CHUNK], F32, tag="kps")
                            nc.tensor.matmul(
                                kps, lhsT=sb["wkT"], rhs=xnT, start=True, stop=True
                            )
                            nc.scalar.activation(
                                out=kT[b][:, c * CHUNK:(c + 1) * CHUNK], in_=kps,
                                func=AFT.Identity, bias=sb["bkcol"],
                            )
                            vps = kvps.tile([CHUNK, D], F32, tag="vps")
                            nc.tensor.matmul(
                                vps, lhsT=xnT, rhs=sb["wvT"], start=True, stop=True
                            )
                            nc.vector.tensor_copy(
                                out=vau[b][:, c, 0:CHUNK], in_=vps
                            )

            # ---------------- slot attention iterations ----------------
            with (
                tc.tile_pool(name="dotps", bufs=3, space="PSUM") as dotps,
                tc.tile_pool(name="updps", bufs=1, space="PSUM") as updps,
                tc.tile_pool(name="smps", bufs=3, space="PSUM") as smps,
                tc.tile_pool(name="wk2", bufs=3) as wk2,
            ):
                P = BPC * NQ  # 16 slot rows across both batches
                slots = wk2.tile([P, D], F32, tag="slots")
                nc.sync.dma_start(out=slots, in_=qry)

                for it in range(ITERS):
                    # q = LN(slots) @ wq.T + bq  (SCALE folded into wq/bq)
                    qln = _layernorm_to(nc, work, eps_t, slots, P, BF16, "ln_q")
                    qlnT_ps = smps.tile([D, P], BF16, tag="sm")
                    nc.tensor.transpose(qlnT_ps, qln, ident[:P, :P])
                    qlnT = work.tile([D, P], BF16, tag="qlnT")
                    nc.scalar.copy(out=qlnT, in_=qlnT_ps)
                    qs_ps = smps.tile([D, P], F32, tag="sm")
                    nc.tensor.matmul(
                        qs_ps, lhsT=sb["wqT"], rhs=qlnT, start=True, stop=True
                    )
                    qsT = work.tile([D, P], BF16, tag="qsT")
                    nc.scalar.activation(
                        out=qsT, in_=qs_ps, func=AFT.Identity, bias=sb["bqcol"]
                    )

                    upd = [
                        updps.tile([NQ, CHUNK + 1], F32, tag=f"upd{b}", name=f"upd{b}")
                        for b in range(BPC)
                    ]
                    for b in range(BPC):
                        for g in range(NGRP):
                            dots = dotps.tile([CHUNK, G * NQ], F32, tag="dots")
                            for u in range(G):
                                c = g * G + u
                                nc.tensor.matmul(
                                    dots[:, u * NQ:(u + 1) * NQ],
                                    lhsT=kT[b][:, c * CHUNK:(c + 1) * CHUNK],
                                    rhs=qsT[:, b * NQ:(b + 1) * NQ],
                                    start=True, stop=True,
                                )
                            smp = wk2.tile([CHUNK, G * NQ], F32, tag="smp")
                            nc.scalar.activation(out=smp, in_=dots, func=AFT.Exp)
                            smp3 = smp.rearrange("p (g i) -> p g i", i=NQ)
                            rs = wk2.tile([CHUNK, G], F32, tag="rs")
                            nc.vector.reduce_sum(out=rs, in_=smp3, axis=AX.X)
                            nc.vector.reciprocal(out=rs, in_=rs)
                            smb = wk2.tile([CHUNK, G, NQ], BF16, tag="smb")
                            rsap = rs[:, :]
                            rs_b = bass.AP(
                                tensor=rsap.tensor, offset=rsap.offset,
                                ap=list(rsap.ap) + [[0, NQ]],
                            )
                            nc.vector.tensor_tensor(
                                out=smb, in0=smp3, in1=rs_b, op=ALU.mult
                            )
                            for u in range(G):
                                c = g * G + u
                                nc.tensor.matmul(
                                    upd[b],
                                    lhsT=smb[:, u, :],
                                    rhs=vau[b][:, c, :],
                                    start=(c == 0), stop=(c == NCH - 1),
                                )

                    # normalize updates by the accumulated softmax denominators
                    updT_ps = smps.tile([D, P], BF16, tag="sm")
                    for b in range(BPC):
                        si = work.tile([NQ, 1], F32, tag="si")
                        nc.vector.reciprocal(out=si, in_=upd[b][:, CHUNK:CHUNK + 1])
                        un = work.tile([NQ, D], BF16, tag=f"un{b}")
                        nc.vector.tensor_scalar_mul(
                            out=un, in0=upd[b][:, 0:CHUNK], scalar1=si
                        )
                        nc.vector.tensor_add(
                            out=un, in0=un, in1=sb["bvrep"][0:NQ, :]
                        )
                        nc.tensor.transpose(
                            updT_ps[:, b * NQ:(b + 1) * NQ], un, ident[:NQ, :NQ]
                        )
                    updT = work.tile([D, P], BF16, tag="updT")
                    nc.scalar.copy(out=updT, in_=updT_ps)

                    # GRU cell (r, z, n), biases folded host-side
                    slots_bf = work.tile([P, D], BF16, tag="slots_bf")
                    nc.vector.tensor_copy(out=slots_bf, in_=slots)
                    sT_ps = smps.tile([D, P], BF16, tag="sm")
                    nc.tensor.transpose(sT_ps, slots_bf, ident[:P, :P])
                    sT = work.tile([D, P], BF16, tag="sT")
                    nc.scalar.copy(out=sT, in_=sT_ps)
                    # r/z thirds of gi+gh accumulate in one PSUM group
                    grz_ps = smps.tile([P, 2 * D], F32, tag="sm")
                    nc.tensor.matmul(grz_ps, lhsT=updT, rhs=sb["wiT"][:, 0:2 * D],
                                     start=True, stop=False)
                    nc.tensor.matmul(grz_ps, lhsT=sT, rhs=sb["whT"][:, 0:2 * D],
                                     start=False, stop=True)
                    gin_ps = smps.tile([P, D], F32, tag="sm")
                    nc.tensor.matmul(gin_ps, lhsT=updT, rhs=sb["wiT"][:, 2 * D:],
                                     start=True, stop=True)
                    ghn_ps = smps.tile([P, D], F32, tag="sm")
                    nc.tensor.matmul(ghn_ps, lhsT=sT, rhs=sb["whT"][:, 2 * D:],
                                     start=True, stop=True)
                    rz = work.tile([P, 2 * D], F32, tag="rz")
                    nc.vector.tensor_add(out=rz, in0=grz_ps, in1=sb["big8"])
                    nc.scalar.activation(out=rz, in_=rz, func=AFT.Sigmoid)
                    hn = work.tile([P, D], F32, tag="hn")
                    nc.vector.tensor_add(out=hn, in0=ghn_ps, in1=sb["bhn8"])
                    nc.vector.tensor_mul(out=hn, in0=hn, in1=rz[:, 0:D])
                    npre = work.tile([P, D], F32, tag="npre")
                    nc.vector.tensor_add(out=npre, in0=gin_ps, in1=sb["bin8"])
                    nc.vector.tensor_add(out=npre, in0=npre, in1=hn)
                    nc.scalar.activation(out=npre, in_=npre, func=AFT.Tanh)
                    tzs = work.tile([P, D], F32, tag="tzs")
                    nc.vector.tensor_sub(out=tzs, in0=slots, in1=npre)
                    nc.vector.tensor_mul(out=tzs, in0=tzs, in1=rz[:, D:2 * D])
                    slots2 = wk2.tile([P, D], F32, tag="slots2")
                    nc.vector.tensor_add(out=slots2, in0=npre, in1=tzs)

                    # feed-forward: slots2 + relu(LN(slots2) @ w1.T + b1) @ w2.T + b2
                    ln2 = _layernorm_to(nc, work, eps_t, slots2, P, BF16, "ln_ff")
                    ln2T_ps = smps.tile([D, P], BF16, tag="sm")
                    nc.tensor.transpose(ln2T_ps, ln2, ident[:P, :P])
                    ln2T = work.tile([D, P], BF16, tag="ln2T")
                    nc.scalar.copy(out=ln2T, in_=ln2T_ps)
                    h1_ps = smps.tile([P, H], F32, tag="sm")
                    nc.tensor.matmul(h1_ps, lhsT=ln2T, rhs=sb["w1T"],
                                     start=True, stop=True)
                    h1a = work.tile([P, H], F32, tag="h1a")
                    nc.vector.tensor_add(out=h1a, in0=h1_ps, in1=sb["b18"])
                    h1 = work.tile([P, H], BF16, tag="h1")
                    nc.scalar.activation(out=h1, in_=h1a, func=AFT.Relu)
                    h1T_ps = smps.tile([D, 4 * P], BF16, tag="sm")
                    for t4 in range(4):
                        nc.tensor.transpose(
                            h1T_ps[:, t4 * P:(t4 + 1) * P],
                            h1[:, t4 * D:(t4 + 1) * D], ident[:P, :P]
                        )
                    h1T = work.tile([D, 4 * P], BF16, tag="h1T")
                    nc.scalar.copy(out=h1T, in_=h1T_ps)
                    ff_ps = smps.tile([P, D], F32, tag="sm")
                    for t4 in range(4):
                        nc.tensor.matmul(
                            ff_ps, lhsT=h1T[:, t4 * P:(t4 + 1) * P],
                            rhs=sb["w2T"][:, t4, :],
                            start=(t4 == 0), stop=(t4 == 3),
                        )
                    slots3 = wk2.tile([P, D], F32, tag="slots")
                    nc.vector.tensor_add(out=slots3, in0=ff_ps, in1=sb["b28"])
                    nc.vector.tensor_add(out=slots3, in0=slots3, in1=slots2)
                    slots = slots3

                nc.sync.dma_start(out=out, in_=slots)
    nc.compile()
    return nc


_CACHE = {}


def _host_params(inputs):
    f8 = np.float64
    wq = np.asarray(inputs["wq"], f8); bq = np.asarray(inputs["bq"], f8)
    wk = np.asarray(inputs["wk"], f8); bk = np.asarray(inputs["bk"], f8)
    wv = np.asarray(inputs["wv"], f8); bv = np.asarray(inputs["bv"], f8)
    gwi = np.asarray(inputs["gru_wi"], f8); gwh = np.asarray(inputs["gru_wh"], f8)
    gbi = np.asarray(inputs["gru_bi"], f8); gbh = np.asarray(inputs["gru_bh"], f8)
    w1 = np.asarray(inputs["mlp_w1"], f8); b1 = np.asarray(inputs["mlp_b1"], f8)
    w2 = np.asarray(inputs["mlp_w2"], f8); b2 = np.asarray(inputs["mlp_b2"], f8)
    liw = np.asarray(inputs["ln_in_w"], f8); lib = np.asarray(inputs["ln_in_b"], f8)
    lqw = np.asarray(inputs["ln_q_w"], f8); lqb = np.asarray(inputs["ln_q_b"], f8)
    lfw = np.asarray(inputs["ln_ff_w"], f8); lfb = np.asarray(inputs["ln_ff_b"], f8)

    wk_f = wk * liw[None, :]; bk_f = bk + lib @ wk.T
    wv_f = wv * liw[None, :]; bv_f = bv + lib @ wv.T
    wq_s = wq * lqw[None, :] * SCALE; bq_s = (bq + lqb @ wq.T) * SCALE
    w1_f = w1 * lfw[None, :]; b1_f = b1 + lfb @ w1.T

    P = BPC * NQ
    return {
        "wkT": np.ascontiguousarray(wk_f.T).astype(NPBF16),
        "wvT": np.ascontiguousarray(wv_f.T).astype(NPBF16),
        "wqT": np.ascontiguousarray(wq_s.T).astype(NPBF16),
        "wiT": np.ascontiguousarray(gwi.T).astype(NPBF16),
        "whT": np.ascontiguousarray(gwh.T).astype(NPBF16),
        "w1T": np.ascontiguousarray(w1_f.T).astype(NPBF16),
        "w2T": np.ascontiguousarray(
            w2.T.reshape(4, D, D).transpose(1, 0, 2)).astype(NPBF16),
        "bkcol": bk_f.reshape(D, 1).astype(np.float32),
        "bqcol": bq_s.reshape(D, 1).astype(np.float32),
        "bvrep": np.tile(bv_f, (D, 1)).astype(np.float32),
        "big8": np.tile(gbi[:2 * D] + gbh[:2 * D], (P, 1)).astype(np.float32),
        "bin8": np.tile(gbi[2 * D:], (P, 1)).astype(np.float32),
        "bhn8": np.tile(gbh[2 * D:], (P, 1)).astype(np.float32),
        "b18": np.tile(b1_f, (P, 1)).astype(np.float32),
        "b28": np.tile(b2, (P, 1)).astype(np.float32),
    }


def kernel(**inputs):
    if "nc" not in _CACHE:
        _CACHE["nc"] = _build_bass()
    nc = _CACHE["nc"]

    params = _host_params(inputs)
    full_inp = np.ascontiguousarray(np.asarray(inputs["inputs"], np.float32))
    full_qry = np.ascontiguousarray(np.asarray(inputs["queries"], np.float32))

    in_maps = []
    for c in range(NCORES):
        m = dict(params)
        m["inp"] = np.ascontiguousarray(full_inp[c * BPC:(c + 1) * BPC])
        m["qry"] = np.ascontiguousarray(
            full_qry[c * BPC:(c + 1) * BPC].reshape(BPC * NQ, D))
        in_maps.append(m)

    res = run_bass_kernel_spmd(nc, in_maps, core_ids=list(range(NCORES)))
    out = np.concatenate(
        [r["out"].reshape(BPC, NQ, D) for r in res.results], axis=0
    )
    return out.astype(np.float32)
